# revision 1
# baseline (speedup 1.0000x reference)
"""Trainium2 Bass kernel for nn_BaseConvFFF (soft-routed conv mixture-of-experts).

Sharding: expert-parallel — each of the 8 cores computes 2 of the 16 leaves
(full batch), plus the full routing scores; host sums the 8 partial
mixture-weighted outputs.

Per-core device program:
  conv1 (3->64ch, 5x5 SAME) as one K=75 im2col matmul per 512-px tile
  routing convs (4 filters) ride the same im2col
  2x2 maxpool + relu fused into PSUM eviction (DVE), written into padded
  per-leaf planes with a +1-shifted copy in partitions 64:127 (K-pair packing)
  conv2 (64->64ch, 5x5 SAME) as 10 K=128 pair + 5 K=64 single matmuls per tile
  global spatial max (DVE reduce) -> 2-layer MLP (matmuls) -> mixture weighting
"""

import sys

if "/opt/trn_rl_repo" not in sys.path:
    sys.path.append("/opt/trn_rl_repo")

import numpy as np

B, CIN = 32, 3
NCORES = 8
HP = 36  # padded pooled plane (32 + 2*2)
HPROWS = 37  # +1 guard row for the shifted upper half
HPAD = 68  # padded conv1 input plane (64 + 2*2)
XPLANE = HPAD * HPAD  # 4624
XPADF = B * XPLANE + 64  # flat padded planes per channel + overrun tail
IMW = 64 * HPAD  # 4352: one im2col row (64 rows x 68, contiguous source)
OUT_W = 100

_cache = {}


def _build(opts=None):
    import concourse.bass as bass
    import concourse.tile as tile
    from concourse import bacc, mybir

    f32 = mybir.dt.float32
    f32r = mybir.dt.float32r
    MAX = mybir.AluOpType.max
    MULT = mybir.AluOpType.mult
    ADD = mybir.AluOpType.add
    AX = mybir.AxisListType.X
    ts = bass.ts

    o = dict(pool1=True, psc=2, psr=1, psd=2, imcol=3, tmpb=3, rtsb=2, slots=2, hq=False, skiprt=False, skipc2=False, skippool=False, skiprtp=False, ev='mix', rtcol=False, psr2=True, psd2=False)
    if opts:
        o.update(opts)
    nc = bacc.Bacc("TRN2", target_bir_lowering=False, debug=False, num_devices=NCORES)

    def din(name, shape, dt):
        return nc.dram_tensor(name, list(shape), dt, kind="ExternalInput").ap()

    xpadf = din("xpadf", (CIN, XPADF), f32r)
    w1T = din("w1T", (75, 128), f32r)
    rw = din("rw", (75, 4), f32r)
    cw2p = din("cw2p", (128, 2, 5, 2, 64), f32r)
    cw2s1 = din("cw2s1", (64, 2, 5, 64), f32r)
    cw2q = din("cw2q", (128, 2, 2, 64), f32r)
    w1sT = din("w1sT", (64, 2, 128), f32r)
    w2sT = din("w2sT", (128, 2, 100), f32r)
    b1sT = din("b1sT", (128, 2), f32)
    b2p = din("b2p", (2, 100), f32r)
    rbias = din("rbias", (4, 1), f32)
    alf = din("alf", (4, 1), f32)
    bet = din("bet", (4, 1), f32)
    hpz = din("hpz", (128, 4, HPROWS, HP), f32r)
    out = nc.dram_tensor("out", [B, OUT_W], f32, kind="ExternalOutput").ap()

    with tile.TileContext(nc) as tc:
        with (
            tc.tile_pool(name="const", bufs=1) as cp,
            tc.tile_pool(name="pers", bufs=1) as pers,
        ):
            w1T_t = cp.tile([75, 128], f32r)
            nc.sync.dma_start(w1T_t[:], w1T)
            rw_t = cp.tile([75, 4], f32r)
            nc.sync.dma_start(rw_t[:], rw)
            cw2p_t = cp.tile([128, 2, 5, 2, 64], f32r)
            nc.sync.dma_start(cw2p_t[:], cw2p)
            cw2s1_t = cp.tile([64, 2, 5, 64], f32r)
            nc.sync.dma_start(cw2s1_t[:], cw2s1)
            cw2q_t = cp.tile([128, 2, 2, 64], f32r)
            nc.sync.dma_start(cw2q_t[:], cw2q)
            w1sT_t = cp.tile([64, 2, 128], f32r)
            nc.sync.dma_start(w1sT_t[:], w1sT)
            w2sT_t = cp.tile([128, 2, 100], f32r)
            nc.sync.dma_start(w2sT_t[:], w2sT)
            b1sT_t = cp.tile([128, 2], f32)
            nc.sync.dma_start(b1sT_t[:], b1sT)
            b2p_t = cp.tile([2, 100], f32r)
            nc.sync.dma_start(b2p_t[:], b2p)
            rbias_t = cp.tile([4, 1], f32)
            nc.sync.dma_start(rbias_t[:], rbias)
            alf_t = cp.tile([4, 1], f32)
            nc.sync.dma_start(alf_t[:], alf)
            bet_t = cp.tile([4, 1], f32)
            nc.sync.dma_start(bet_t[:], bet)

            # persistent working buffers
            hp0 = pers.tile([128, o["slots"], HPROWS, HP], f32r)
            hp1 = pers.tile([128, o["slots"], HPROWS, HP], f32r)
            nc.sync.dma_start(hp0[:], hpz[:, : o["slots"]])
            nc.sync.dma_start(hp1[:], hpz[:, : o["slots"]])
            hps = (hp0, hp1)
            hq0 = pers.tile([128, o["slots"], HPROWS, HP], f32r)
            hq1 = pers.tile([128, o["slots"], HPROWS, HP], f32r)
            nc.sync.dma_start(hq0[:], hpz[:, : o["slots"]])
            nc.sync.dma_start(hq1[:], hpz[:, : o["slots"]])
            hqs = (hq0, hq1)
            featsc0 = pers.tile([64, 2 * B], f32)
            featsc1 = pers.tile([64, 2 * B], f32)
            featscs = (featsc0, featsc1)
            rtsc = pers.tile([128, B], f32)

            with (
                tc.tile_pool(name="imcol", bufs=o["imcol"]) as impool,
                tc.tile_pool(name="rts", bufs=o["rtsb"]) as rtspool,
                tc.tile_pool(name="tmp", bufs=o["tmpb"]) as tmppool,
                tc.tile_pool(name="rtp", bufs=2) as rtppool,
                tc.tile_pool(name="psc", bufs=o["psc"], space="PSUM") as pscp,
                tc.tile_pool(name="psr", bufs=o["psr"], space="PSUM") as psrp,
                tc.tile_pool(name="psd", bufs=o["psd"], space="PSUM") as psdp,
            ):
                for b in range(B):
                    slot = b % o["slots"]
                    # ---- im2col: partition p=(c,dy,dx) holds the padded
                    # plane shifted by (dy,dx) — contiguous 4352-elem source
                    imc = impool.tile([75, IMW], f32r)
                    for c in range(CIN):
                        src = bass.AP(
                            xpadf.tensor,
                            c * XPADF + b * XPLANE,
                            [[HPAD, 5], [1, 5], [1, IMW]],
                        )
                        nc.sync.dma_start(imc[c * 25 : (c + 1) * 25, :], src)
                    imcv = imc.rearrange("p (y x) -> p y x", y=64, x=HPAD)

                    rts = rtspool.tile([4, 4096], f32)
                    rtsv = rts
                    for pair in range(4):
                        if o["psr2"]:
                            psR2 = psrp.tile([4, 2, 512], f32, name="psR2")
                        # conv1: 128 out-ch (2 leaves), K=75 matmuls into a
                        # 2-bank psum tile (two 512-px halves)
                        psC2 = pscp.tile([128, 2, 512], f32)
                        for h in range(2):
                            t = 2 * pair + h
                            rhs = imcv[:, 8 * t : 8 * t + 8, 0:64]
                            nc.tensor.matmul(
                                psC2[:, h, :], w1T_t[:], rhs,
                                start=True, stop=True,
                            )
                            # routing: 4 filters, same rhs
                            if o["skiprt"]:
                                pass
                            elif o["psr2"]:
                                nc.tensor.matmul(
                                    psR2[:, h, :], rw_t[:], rhs,
                                    start=True, stop=True,
                                )
                            elif o["rtcol"]:
                                psRb = psrp.tile([128, 512], f32, name="psRb")
                                psRv = psRb[64:68, :]
                                nc.tensor.matmul(
                                    psRv, rw_t[:], rhs,
                                    start=True, stop=True, tile_position=(0, 64),
                                )
                            else:
                                psR = psrp.tile([4, 512], f32)
                                psRv = psR[:]
                                nc.tensor.matmul(
                                    psRv, rw_t[:], rhs,
                                    start=True, stop=True,
                                )
                            if not o["skiprt"] and not o["psr2"]:
                                nc.scalar.activation(
                                    rts[:, ts(t, 512)], psRv,
                                    mybir.ActivationFunctionType.Copy,
                                )
                        if not o["skiprt"] and o["psr2"]:
                            nc.scalar.activation(
                                rts[:, ts(pair, 1024)], psR2[:],
                                mybir.ActivationFunctionType.Copy,
                            )
                        # maxpool 2x2 + relu eviction (both halves at once)
                        y0 = 2 + 8 * pair
                        if o["skippool"]:
                            pass
                        elif o["pool1"]:
                            # one 5D reduce does the whole 2x2 pool
                            pcv = psC2.rearrange(
                                "p h (yb wy x wx) -> p (h yb) x wy wx",
                                yb=4, wy=2, x=32, wx=2,
                            )
                            tx = tmppool.tile([128, 8, 32], f32)
                            nc.vector.tensor_reduce(
                                tx[:], pcv[:], axis=mybir.AxisListType.XY, op=MAX
                            )
                            for leaf in range(2):
                                hp = hps[leaf]
                                th = tx[64 * leaf : 64 * leaf + 64, :, :]
                                if o["ev"] == "act":
                                    nc.scalar.activation(
                                        hp[0:64, slot, y0 : y0 + 8, 2:34], th,
                                        mybir.ActivationFunctionType.Relu,
                                    )
                                else:
                                    wr = (
                                        nc.gpsimd if o["ev"] == "gp" else nc.vector
                                    )
                                    wr.tensor_scalar_max(
                                        hp[0:64, slot, y0 : y0 + 8, 2:34], th, 0.0
                                    )
                                if o["ev"] in ("gp", "dve4"):
                                    wr.tensor_scalar_max(
                                        hp[64:128, slot, y0 : y0 + 8, 1:33], th, 0.0
                                    )
                                elif o["ev"] in ("mix", "act"):
                                    nc.scalar.activation(
                                        hp[64:128, slot, y0 : y0 + 8, 1:33], th,
                                        mybir.ActivationFunctionType.Relu,
                                    )
                        else:
                            pcv = psC2.rearrange(
                                "p h (y x t) -> p (h y) x t", y=8, x=32, t=2
                            )
                            tx = tmppool.tile([128, 8, 2, 32], f32)
                            txv = tx.rearrange("p a b x -> p (a b) x")
                            nc.vector.reduce_max(txv[:], pcv[:], axis=AX)
                            for leaf in range(2):
                                hp = hps[leaf]
                                dst = hp[0:64, slot, y0 : y0 + 8, 2:34]
                                nc.vector.scalar_tensor_tensor(
                                    dst,
                                    tx[64 * leaf : 64 * leaf + 64, :, 0, :],
                                    0.0,
                                    tx[64 * leaf : 64 * leaf + 64, :, 1, :],
                                    op0=MAX, op1=MAX,
                                )
                                # shifted (+1 elem) copy for conv2 K-pair packing
                                nc.gpsimd.tensor_copy(
                                    hp[64:128, slot, y0 : y0 + 8, 1:33], dst
                                )

                    if o["ev"] == "dma":
                        for leaf in range(2):
                            hp = hps[leaf]
                            nc.sync.dma_start(
                                hp[64:128, slot, 2:34, 1:33],
                                hp[0:64, slot, 2:34, 2:34],
                            )
                    if o["skiprt"]:
                        nc.vector.memset(rts[:], 0.0)
                    if o["hq"]:
                        # hq: lower = relu'd pooled plane, upper = same shifted
                        # one row up (+36) -> dy-pairs for the dx=4 taps
                        for leaf in range(2):
                            hp, hq = hps[leaf], hqs[leaf]
                            nc.sync.dma_start(
                                hq[0:64, slot, 2:34, 2:34],
                                hp[0:64, slot, 2:34, 2:34],
                            )
                            nc.sync.dma_start(
                                hq[64:128, slot, 1:33, 2:34],
                                hp[0:64, slot, 2:34, 2:34],
                            )

                    # routing per-image: repartition [4,4096] -> [(d j), 128]
                    if not o["skiprtp"]:
                        rtp = rtppool.tile([128, 128], f32)
                        nc.sync.dma_start(rtp[:], rts.rearrange("d (j e) -> d j e", j=32))
                        nc.vector.reduce_max(rtsc[:, b : b + 1], rtp[:], axis=AX)

                    # ---- conv2 per leaf: 10 K=128 dx-pairs + dx=4 taps
                    for leaf in range(2) if not o["skipc2"] else []:
                        hp = hps[leaf]
                        if o["psd2"]:
                            psD2 = psdp.tile([64, 2, 512], f32, name="psD2")
                        for t2 in range(2):
                            psD = psD2[:, t2, :] if o["psd2"] else psdp.tile([64, 512], f32)
                            first = True
                            for dy in range(5):
                                for j in range(2):
                                    nc.tensor.matmul(
                                        psD[:],
                                        cw2p_t[:, leaf, dy, j, :],
                                        hp[:, slot, 16 * t2 + dy : 16 * t2 + dy + 16,
                                           2 * j : 2 * j + 32],
                                        start=first, stop=False,
                                    )
                                    first = False
                                if not o["hq"]:
                                    nc.tensor.matmul(
                                        psD[:],
                                        cw2s1_t[:, leaf, dy, :],
                                        hp[0:64, slot, 16 * t2 + dy : 16 * t2 + dy + 16,
                                           4:36],
                                        start=False, stop=(dy == 4),
                                    )
                            if o["hq"]:
                                hq = hqs[leaf]
                                for q in range(2):
                                    nc.tensor.matmul(
                                        psD[:],
                                        cw2q_t[:, leaf, q, :],
                                        hq[:, slot,
                                           16 * t2 + 2 * q : 16 * t2 + 2 * q + 16,
                                           4:36],
                                        start=False, stop=False,
                                    )
                                nc.tensor.matmul(
                                    psD[:],
                                    cw2s1_t[:, leaf, 4, :],
                                    hp[0:64, slot, 16 * t2 + 4 : 16 * t2 + 4 + 16,
                                       4:36],
                                    start=False, stop=True,
                                )
                            if not o["psd2"]:
                                nc.vector.reduce_max(
                                    featscs[leaf][:, 2 * b + t2 : 2 * b + t2 + 1],
                                    psD[:], axis=AX,
                                )
                        if o["psd2"]:
                            nc.vector.reduce_max(
                                featscs[leaf][:, b : b + 1],
                                psD2[:].rearrange("p q n -> p (q n)"), axis=AX,
                            )

            # ---------------- finalize: routing mix + MLP ----------------
            with (
                tc.tile_pool(name="fin", bufs=1) as fin,
                tc.tile_pool(name="psm", bufs=1, space="PSUM") as psm,
            ):
                rtj = fin.tile([4, 32, B], f32)
                nc.sync.dma_start(rtj[:], rtsc[:])
                scoresT = fin.tile([4, B], f32)
                nc.vector.reduce_max(
                    scoresT[:], rtj.rearrange("d j b -> d b j"), axis=AX
                )
                sg = fin.tile([4, B], f32)
                nc.scalar.activation(
                    sg[:], scoresT[:], mybir.ActivationFunctionType.Sigmoid,
                    bias=rbias_t[:, 0:1],
                )
                fsel = fin.tile([4, B], f32)
                nc.vector.tensor_scalar(
                    fsel[:], sg[:], alf_t[:, 0:1], bet_t[:, 0:1], op0=MULT, op1=ADD
                )
                fT = fin.tile([B, 4], f32)
                for d in range(4):
                    nc.sync.dma_start(fT[:, d : d + 1], fsel[d : d + 1, :])
                t01 = fin.tile([B, 1], f32)
                nc.vector.tensor_mul(t01[:], fT[:, 0:1], fT[:, 1:2])
                m012 = fin.tile([B, 1], f32)
                nc.vector.tensor_mul(m012[:], t01[:], fT[:, 2:3])
                mixpair = fin.tile([B, 2], f32)
                nc.vector.tensor_mul(mixpair[:, 1:2], m012[:], fT[:, 3:4])
                nc.vector.tensor_sub(mixpair[:, 0:1], m012[:], mixpair[:, 1:2])
                mixpR = fin.tile([B, 2], f32r)
                nc.vector.tensor_copy(mixpR[:], mixpair[:])
                mixT = fin.tile([2, B], f32r)
                for leaf in range(2):
                    nc.sync.dma_start(
                        mixT[leaf : leaf + 1, :], mixpR[:, leaf : leaf + 1]
                    )

                if o["skipc2"]:
                    nc.vector.memset(featsc0[:], 0.0)
                    nc.vector.memset(featsc1[:], 0.0)
                if o["skiprtp"]:
                    nc.vector.memset(rtsc[:], 0.0)
                ps2s = []
                for leaf in range(2):
                    featT = fin.tile([64, B], f32r, name=f"featT{leaf}")
                    if o["psd2"]:
                        nc.vector.tensor_scalar_max(
                            featT[:], featscs[leaf][:, 0:B], 0.0
                        )
                    else:
                        nc.vector.reduce_max(
                            featT[:],
                            featscs[leaf].rearrange("p (b t) -> p b t", t=2),
                            axis=AX,
                        )
                        nc.vector.tensor_scalar_max(featT[:], featT[:], 0.0)
                    ps1 = psm.tile([128, B], f32, name=f"ps1_{leaf}")
                    nc.tensor.matmul(
                        ps1[:], w1sT_t[:, leaf, :], featT[:], start=True, stop=True
                    )
                    h1b = fin.tile([128, B], f32r, name=f"h1b{leaf}")
                    nc.vector.tensor_scalar_add(
                        h1b[:], ps1[:], b1sT_t[:, leaf : leaf + 1]
                    )
                    ps2 = psm.tile([B, OUT_W], f32, name=f"ps2_{leaf}")
                    nc.tensor.matmul(
                        ps2[:], h1b[:], w2sT_t[:, leaf, :], start=True, stop=True
                    )
                    ps2s.append(ps2)

                psb = psm.tile([B, OUT_W], f32)
                nc.tensor.matmul(psb[:], mixT[:], b2p_t[:], start=True, stop=True)

                acc = fin.tile([B, OUT_W], f32)
                nc.vector.tensor_scalar_mul(acc[:], ps2s[0][:], mixpair[:, 0:1])
                acc2 = fin.tile([B, OUT_W], f32)
                nc.vector.scalar_tensor_tensor(
                    acc2[:], ps2s[1][:], mixpair[:, 1:2], acc[:], op0=MULT, op1=ADD
                )
                osb = fin.tile([B, OUT_W], f32)
                nc.vector.tensor_add(osb[:], acc2[:], psb[:])
                nc.sync.dma_start(out, osb[:])

    nc.compile()
    return nc


def host_pack(inputs, core):
    x = np.ascontiguousarray(np.asarray(inputs["x"], np.float32))
    node_weights = np.asarray(inputs["node_weights"], np.float32)
    node_biases = np.asarray(inputs["node_biases"], np.float32)
    cw1s = np.asarray(inputs["cw1s"], np.float32)
    cw2s = np.asarray(inputs["cw2s"], np.float32)
    w1s = np.asarray(inputs["w1s"], np.float32)
    b1s = np.asarray(inputs["b1s"], np.float32)
    w2s = np.asarray(inputs["w2s"], np.float32)
    b2s = np.asarray(inputs["b2s"], np.float32)

    l0 = 2 * core
    xpad = np.zeros((CIN, B, HPAD, HPAD), np.float32)
    xpad[:, :, 2:66, 2:66] = x.transpose(1, 0, 2, 3)
    xpadf = np.zeros((CIN, XPADF), np.float32)
    xpadf[:, : B * XPLANE] = xpad.reshape(CIN, -1)

    # conv1 lhsT (75, 128): row p=(c,dy,dx), col m=(leaf, ch)
    # cw1s[l,ch,c,dy,dx] -> transpose to (c,dy,dx, ch) then reshape
    w1T = np.zeros((75, 128), np.float32)
    for leaf in range(2):
        w1T[:, 64 * leaf : 64 * leaf + 64] = (
            cw1s[l0 + leaf].transpose(1, 2, 3, 0).reshape(75, 64)
        )
    idx = [0, 2, 6, 14]
    rw = node_weights[idx, 0].transpose(1, 2, 3, 0).reshape(75, 4).copy()

    cw2p = np.zeros((128, 2, 5, 2, 64), np.float32)
    cw2s1 = np.zeros((64, 2, 5, 64), np.float32)
    for leaf in range(2):
        w = cw2s[l0 + leaf]  # (m=64, ci=64, dy, dx)
        for dy in range(5):
            for j in range(2):
                cw2p[0:64, leaf, dy, j, :] = w[:, :, dy, 2 * j].T
                cw2p[64:128, leaf, dy, j, :] = w[:, :, dy, 2 * j + 1].T
            cw2s1[:, leaf, dy, :] = w[:, :, dy, 4].T
    cw2q = np.zeros((128, 2, 2, 64), np.float32)
    for leaf in range(2):
        w = cw2s[l0 + leaf]
        for q in range(2):
            cw2q[0:64, leaf, q, :] = w[:, :, 2 * q, 4].T
            cw2q[64:128, leaf, q, :] = w[:, :, 2 * q + 1, 4].T

    w1sT = np.stack([w1s[l0], w1s[l0 + 1]], axis=1)  # (64, 2, 128)
    w2sT = np.stack([w2s[l0], w2s[l0 + 1]], axis=1)  # (128, 2, 100)
    b1sT = np.stack([b1s[l0], b1s[l0 + 1]], axis=1)  # (128, 2)
    b2p = np.stack([b2s[l0], b2s[l0 + 1]], axis=0)  # (2, 100)

    rbias = np.zeros((4, 1), np.float32)
    alfv = np.zeros((4, 1), np.float32)
    betv = np.zeros((4, 1), np.float32)
    for d in range(4):
        plat = 2**d - 1
        g = l0 >> (3 - d)
        j, s = g >> 1, g & 1
        rbias[d, 0] = node_biases[plat + j, 0]
        if d < 3:
            alfv[d, 0], betv[d, 0] = (1.0, 0.0) if s == 1 else (-1.0, 1.0)
        else:
            alfv[d, 0], betv[d, 0] = 1.0, 0.0
    return dict(
        xpadf=xpadf, w1T=w1T, rw=rw, cw2p=cw2p, cw2s1=cw2s1, cw2q=cw2q, w1sT=w1sT,
        w2sT=np.ascontiguousarray(w2sT), b1sT=np.ascontiguousarray(b1sT),
        b2p=np.ascontiguousarray(b2p), rbias=rbias, alf=alfv, bet=betv,
        hpz=np.zeros((128, 4, HPROWS, HP), np.float32),
    )


def kernel(**inputs):
    from concourse import bass_utils

    if "nc" not in _cache:
        _cache["nc"] = _build()
    nc = _cache["nc"]
    in_maps = [host_pack(inputs, c) for c in range(NCORES)]
    res = bass_utils.run_bass_kernel_spmd(nc, in_maps, core_ids=list(range(NCORES)))
    total = np.zeros((B, OUT_W), np.float32)
    for c in range(NCORES):
        total += res.results[c]["out"]
    return total

